# revision 4
# baseline (speedup 1.0000x reference)
"""CorrespondenceGeneration kernel for 8 TRN2 NeuronCores.

Reference computation (per item): unit-normalize features over channels,
build 3x3 patch matrices, corr = inp_patches^T @ ref_patches, argmax over
ref patches (first occurrence on ties), convert argmax index to flow,
9 tensor-shifts, channel reorder.

Sharding: core c -> (item = c//4, n_in chunk = c%4 of 2209 rows). Each core
computes its corr rows against ALL ref patches, streamed in 6 column strips
of 1536; per strip the DVE max/max_index ops produce (top1 val, first idx)
per row. Host combines strips (strict >, ascending strip order == exact
first-occurrence-tie argmax) and does the cheap index->flow postprocessing.

Note: the reference's per-patch-column normalization of ref divides every
column by ||col||+eps with ||col|| == 3 exactly (9 unit-norm pixels), a
global positive scale that argmax is invariant to -- so it is skipped.
"""

import sys

if "/opt/trn_rl_repo" not in sys.path:
    sys.path.insert(0, "/opt/trn_rl_repo")

import numpy as np

# ---- problem constants (hardcoded; kernel.py must be self-contained) ----
N_ITEMS = 2
C = 64
H = W = 96
PS = 3
HP = WP = H - PS + 1          # 94
NPATCH = HP * WP              # 8836
K = C * PS * PS               # 576
KPAD = 640                    # 5 x 128
KCH = 5                       # K chunks of 128
N_CORES = 8
CHUNKS_PER_ITEM = 4
CHUNK = NPATCH // CHUNKS_PER_ITEM      # 2209
CHUNK_PAD = 2304                       # 18 x 128
N_BLOCKS = CHUNK_PAD // 128            # 18
REF_PAD = 9216                         # 6 x 1536
STRIP = 1536                           # 3 PSUM banks of 512 fp32
N_STRIPS = REF_PAD // STRIP            # 6
EPS_NORMALIZE = 1e-12

# matmul input dtype: "float32" (exact, 4 cyc/row) or "float32r" (1 cyc/row)
MM_DTYPE = "float32r"
# float32r matmul error is ~2.6e-4 max (measured); rows whose top-2 corr gap
# is below this threshold get an exact fp64 rescore on the host (~700 rows).
MARGIN_THRESH = 4e-3

_COMPILED = {}


def _build_module():
    import concourse.bacc as bacc
    from concourse.tile import TileContext
    from concourse import mybir

    dt_mm = getattr(mybir.dt, MM_DTYPE)
    nc = bacc.Bacc("TRN2", target_bir_lowering=False, debug=False,
                   num_devices=N_CORES)
    inp_d = nc.dram_tensor("inp", [KCH, 128, CHUNK_PAD], dt_mm,
                           kind="ExternalInput").ap()
    ref_d = nc.dram_tensor("ref", [KCH, 128, REF_PAD], dt_mm,
                           kind="ExternalInput").ap()
    NSLOT = N_BLOCKS * N_STRIPS            # 108
    val_d = nc.dram_tensor("val", [128, NSLOT * 8], mybir.dt.float32,
                           kind="ExternalOutput").ap()
    idx_d = nc.dram_tensor("idx", [128, NSLOT * 8], mybir.dt.uint32,
                           kind="ExternalOutput").ap()

    with TileContext(nc) as tc:
        with tc.tile_pool(name="inp", bufs=1) as inp_pool, \
             tc.tile_pool(name="ref", bufs=3) as ref_pool, \
             tc.tile_pool(name="acc", bufs=1) as acc_pool, \
             tc.tile_pool(name="psum", bufs=2, space="PSUM") as psum_pool:
            inp_sb = inp_pool.tile([128, KCH * CHUNK_PAD], dt_mm)
            for k in range(KCH):
                nc.sync.dma_start(
                    inp_sb[:, k * CHUNK_PAD:(k + 1) * CHUNK_PAD], inp_d[k])
            acc_val = acc_pool.tile([128, NSLOT * 8], mybir.dt.float32)
            acc_idx = acc_pool.tile([128, NSLOT * 8], mybir.dt.uint32)
            for s in range(N_STRIPS):
                ref_sb = ref_pool.tile([128, KCH * STRIP], dt_mm)
                for k in range(KCH):
                    nc.sync.dma_start(
                        ref_sb[:, k * STRIP:(k + 1) * STRIP],
                        ref_d[k, :, s * STRIP:(s + 1) * STRIP])
                for b in range(N_BLOCKS):
                    pt = psum_pool.tile([128, STRIP], mybir.dt.float32)
                    for k in range(KCH):
                        for j in range(STRIP // 512):
                            nc.tensor.matmul(
                                pt[:, j * 512:(j + 1) * 512],
                                inp_sb[:, k * CHUNK_PAD + b * 128:
                                       k * CHUNK_PAD + (b + 1) * 128],
                                ref_sb[:, k * STRIP + j * 512:
                                       k * STRIP + (j + 1) * 512],
                                start=(k == 0), stop=(k == KCH - 1))
                    slot = (b * N_STRIPS + s) * 8
                    nc.vector.max(acc_val[:, slot:slot + 8], pt[:])
                    nc.vector.max_index(acc_idx[:, slot:slot + 8],
                                        acc_val[:, slot:slot + 8], pt[:])
            nc.sync.dma_start(val_d[:], acc_val[:])
            nc.sync.dma_start(idx_d[:], acc_idx[:])

    nc.compile()
    return nc


def _get_nc():
    if "nc" not in _COMPILED:
        _COMPILED["nc"] = _build_module()
    return _COMPILED["nc"]


def _unit_channels(f):
    # f: (N, C, H, W) float32; unit L2 norm over channels per pixel
    n = np.sqrt(np.sum(f * f, axis=1, keepdims=True, dtype=np.float32))
    return (f / np.maximum(n, EPS_NORMALIZE)).astype(np.float32)


def _patches(f):
    # f: (C, H, W) -> (K, NPATCH), row index = c*9 + dy*3 + dx
    out = np.empty((C, PS * PS, HP, WP), np.float32)
    for dy in range(PS):
        for dx in range(PS):
            out[:, dy * PS + dx] = f[:, dy:dy + HP, dx:dx + WP]
    return out.reshape(K, NPATCH)


def _prep_inputs(dense_features1, dense_features2):
    fi = _unit_channels(np.ascontiguousarray(dense_features1, np.float32))
    fr = _unit_channels(np.ascontiguousarray(dense_features2, np.float32))
    in_maps = []
    mats = []
    for n in range(N_ITEMS):
        inp_full = _patches(fi[n])                       # (576, 8836)
        ref_full = _patches(fr[n])                       # (576, 8836)
        mats.append((inp_full, ref_full))
        ref_pad = np.zeros((KPAD, REF_PAD), np.float32)
        ref_pad[:K, :NPATCH] = ref_full
        ref_pad = np.ascontiguousarray(
            ref_pad.reshape(KCH, 128, REF_PAD))
        for j in range(CHUNKS_PER_ITEM):
            inp_pad = np.zeros((KPAD, CHUNK_PAD), np.float32)
            inp_pad[:K, :CHUNK] = inp_full[:, j * CHUNK:(j + 1) * CHUNK]
            inp_pad = np.ascontiguousarray(
                inp_pad.reshape(KCH, 128, CHUNK_PAD))
            in_maps.append({"inp": inp_pad, "ref": ref_pad})
    return in_maps, mats


def _combine_core(val, idx):
    # val/idx: (128, N_BLOCKS*N_STRIPS*8)
    # -> (CHUNK,) global ref argmax, (CHUNK,) top1-top2 margin
    v8 = val.reshape(128, N_BLOCKS, N_STRIPS, 8)
    v8 = v8.transpose(1, 0, 2, 3).reshape(CHUNK_PAD, N_STRIPS * 8)[:CHUNK]
    v = v8[:, ::8].reshape(CHUNK, N_STRIPS)       # per-strip top-1
    ix = idx.reshape(128, N_BLOCKS, N_STRIPS, 8)[..., 0].astype(np.int64)
    ix = ix.transpose(1, 0, 2).reshape(CHUNK_PAD, N_STRIPS)[:CHUNK]
    g = ix + (np.arange(N_STRIPS, dtype=np.int64) * STRIP)[None, :]
    sel = np.argmax(v, axis=1)            # first occurrence == earliest strip
    top2 = np.partition(v8, N_STRIPS * 8 - 2, axis=1)[:, -2:]
    margin = top2[:, 1] - top2[:, 0]
    return g[np.arange(CHUNK), sel], margin


def _flow_output(max_idx):
    # max_idx: (NPATCH,) int -> (18, H, W) float32, mirroring the reference
    mi = max_idx.reshape(HP, WP)
    fw = (mi % WP).astype(np.float32) - np.arange(WP, dtype=np.float32)[None, :]
    fh = (mi // WP).astype(np.float32) - np.arange(HP, dtype=np.float32)[:, None]
    flow = np.stack([fw, fh], axis=-1)                     # (94, 94, 2)
    flow = np.pad(flow, ((0, PS - 1), (0, PS - 1), (0, 0)))  # (96, 96, 2)
    shifted = np.stack([np.pad(flow, ((i, 0), (j, 0), (0, 0)))[:H, :W]
                        for i in range(PS) for j in range(PS)], axis=0)
    out = np.stack([shifted[..., 1], shifted[..., 0]], axis=1)  # (9, 2, H, W)
    return out.reshape(2 * PS * PS, H, W).astype(np.float32)


def kernel(dense_features1, dense_features2):
    from concourse import bass_utils

    nc = _get_nc()
    in_maps, mats = _prep_inputs(dense_features1, dense_features2)
    res = bass_utils.run_bass_kernel_spmd(
        nc, in_maps, core_ids=list(range(N_CORES)))
    out = np.empty((N_ITEMS, 2 * PS * PS, H, W), np.float32)
    for n in range(N_ITEMS):
        parts = [
            _combine_core(res.results[n * CHUNKS_PER_ITEM + j]["val"],
                          res.results[n * CHUNKS_PER_ITEM + j]["idx"])
            for j in range(CHUNKS_PER_ITEM)
        ]
        max_idx = np.concatenate([p[0] for p in parts])
        margin = np.concatenate([p[1] for p in parts])
        flagged = np.flatnonzero(margin < MARGIN_THRESH)
        if flagged.size:
            inp_full, ref_full = mats[n]
            corr = inp_full[:, flagged].T.astype(np.float64) @ \
                ref_full.astype(np.float64)
            max_idx[flagged] = np.argmax(corr, axis=1)
        out[n] = _flow_output(max_idx)
    return out


# revision 6
# speedup vs baseline: 1.1017x; 1.1017x over previous
"""CorrespondenceGeneration kernel for 8 TRN2 NeuronCores.

Reference computation (per item): unit-normalize features over channels,
build 3x3 patch matrices, corr = inp_patches^T @ ref_patches, argmax over
ref patches (first occurrence on ties), convert argmax index to flow,
9 tensor-shifts, channel reorder.

Sharding: core c -> (item = c//4, n_in chunk = c%4 of 2209 rows). Each core
computes its corr rows against ALL ref patches, streamed in 6 column strips
of 1536; per strip the DVE max/max_index ops produce (top1 val, first idx)
per row. Host combines strips (strict >, ascending strip order == exact
first-occurrence-tie argmax) and does the cheap index->flow postprocessing.

Note: the reference's per-patch-column normalization of ref divides every
column by ||col||+eps with ||col|| == 3 exactly (9 unit-norm pixels), a
global positive scale that argmax is invariant to -- so it is skipped.
"""

import sys

if "/opt/trn_rl_repo" not in sys.path:
    sys.path.insert(0, "/opt/trn_rl_repo")

import numpy as np

# ---- problem constants (hardcoded; kernel.py must be self-contained) ----
N_ITEMS = 2
C = 64
H = W = 96
PS = 3
HP = WP = H - PS + 1          # 94
NPATCH = HP * WP              # 8836
K = C * PS * PS               # 576
KPAD = 640                    # 5 x 128
KCH = 5                       # K chunks of 128
N_CORES = 8
CHUNKS_PER_ITEM = 4
CHUNK = NPATCH // CHUNKS_PER_ITEM      # 2209
CHUNK_PAD = 2304                       # 18 x 128
N_BLOCKS = CHUNK_PAD // 128            # 18
REF_PAD = 9216                         # 6 x 1536
STRIP = 1536                           # 3 PSUM banks of 512 fp32
N_STRIPS = REF_PAD // STRIP            # 6
EPS_NORMALIZE = 1e-12

# matmul input dtype: "float32" (exact, 4 cyc/row) or "float32r" (1 cyc/row)
MM_DTYPE = "float32r"
# float32r matmul error is ~2.6e-4 max (measured); rows whose top-2 corr gap
# is below this threshold get an exact fp64 rescore on the host (~700 rows).
MARGIN_THRESH = 4e-3

_COMPILED = {}


def _build_module():
    import concourse.bacc as bacc
    from concourse.tile import TileContext
    from concourse import mybir

    dt_mm = getattr(mybir.dt, MM_DTYPE)
    nc = bacc.Bacc("TRN2", target_bir_lowering=False, debug=False,
                   num_devices=N_CORES)
    inp_d = nc.dram_tensor("inp", [KCH, 128, CHUNK_PAD], dt_mm,
                           kind="ExternalInput").ap()
    ref_d = nc.dram_tensor("ref", [KCH, 128, REF_PAD], dt_mm,
                           kind="ExternalInput").ap()
    NSLOT = N_BLOCKS * N_STRIPS            # 108
    val_d = nc.dram_tensor("val", [128, NSLOT * 8], mybir.dt.float32,
                           kind="ExternalOutput").ap()
    idx_d = nc.dram_tensor("idx", [128, NSLOT * 8], mybir.dt.uint32,
                           kind="ExternalOutput").ap()

    # last strip: only NPATCH - 5*STRIP = 1156 real columns for the DVE scan
    # (PE still computes the zero-padded full strip; a 132-wide matmul would
    # fall off float32r's fast path below N=256 and cost more than it saves)
    tail = NPATCH - (N_STRIPS - 1) * STRIP

    with TileContext(nc) as tc:
        with tc.tile_pool(name="inp", bufs=1) as inp_pool, \
             tc.tile_pool(name="ref", bufs=3) as ref_pool, \
             tc.tile_pool(name="corr", bufs=4) as corr_pool, \
             tc.tile_pool(name="acc", bufs=1) as acc_pool, \
             tc.tile_pool(name="psum", bufs=2, space="PSUM") as psum_pool:
            # first ref strip before the big inp transfer so PE starts sooner
            ref_tiles = {}
            ref_tiles[0] = ref_pool.tile([128, KCH * STRIP], dt_mm, tag="ref", name="ref_sb0")
            for k in range(KCH):
                nc.sync.dma_start(
                    ref_tiles[0][:, k * STRIP:(k + 1) * STRIP],
                    ref_d[k, :, 0:STRIP])
            inp_sb = inp_pool.tile([128, KCH * CHUNK_PAD], dt_mm)
            for k in range(KCH):
                nc.sync.dma_start(
                    inp_sb[:, k * CHUNK_PAD:(k + 1) * CHUNK_PAD], inp_d[k])
            acc_val = acc_pool.tile([128, NSLOT * 8], mybir.dt.float32)
            acc_idx = acc_pool.tile([128, NSLOT * 8], mybir.dt.uint32)
            for s in range(N_STRIPS):
                if s not in ref_tiles:
                    ref_tiles[s] = ref_pool.tile([128, KCH * STRIP], dt_mm,
                                                 tag="ref", name=f"ref_sb{s}")
                    for k in range(KCH):
                        nc.sync.dma_start(
                            ref_tiles[s][:, k * STRIP:(k + 1) * STRIP],
                            ref_d[k, :, s * STRIP:(s + 1) * STRIP])
                ref_sb = ref_tiles[s]
                width = STRIP if s < N_STRIPS - 1 else tail
                for b in range(N_BLOCKS):
                    pt = psum_pool.tile([128, STRIP], mybir.dt.float32)
                    for k in range(KCH):
                        for j in range(STRIP // 512):
                            nc.tensor.matmul(
                                pt[:, j * 512:(j + 1) * 512],
                                inp_sb[:, k * CHUNK_PAD + b * 128:
                                       k * CHUNK_PAD + (b + 1) * 128],
                                ref_sb[:, k * STRIP + j * 512:
                                       k * STRIP + (j + 1) * 512],
                                start=(k == 0), stop=(k == KCH - 1))
                    ct = corr_pool.tile([128, STRIP], mybir.dt.float32)
                    nc.scalar.copy(ct[:], pt[:])
                    slot = (b * N_STRIPS + s) * 8
                    nc.vector.max(acc_val[:, slot:slot + 8], ct[:, :width])
                    nc.vector.max_index(acc_idx[:, slot:slot + 8],
                                        acc_val[:, slot:slot + 8],
                                        ct[:, :width])
            nc.sync.dma_start(val_d[:], acc_val[:])
            nc.sync.dma_start(idx_d[:], acc_idx[:])

    nc.compile()
    return nc


def _get_nc():
    if "nc" not in _COMPILED:
        _COMPILED["nc"] = _build_module()
    return _COMPILED["nc"]


def _unit_channels(f):
    # f: (N, C, H, W) float32; unit L2 norm over channels per pixel
    n = np.sqrt(np.sum(f * f, axis=1, keepdims=True, dtype=np.float32))
    return (f / np.maximum(n, EPS_NORMALIZE)).astype(np.float32)


def _patches(f):
    # f: (C, H, W) -> (K, NPATCH), row index = c*9 + dy*3 + dx
    out = np.empty((C, PS * PS, HP, WP), np.float32)
    for dy in range(PS):
        for dx in range(PS):
            out[:, dy * PS + dx] = f[:, dy:dy + HP, dx:dx + WP]
    return out.reshape(K, NPATCH)


def _prep_inputs(dense_features1, dense_features2):
    fi = _unit_channels(np.ascontiguousarray(dense_features1, np.float32))
    fr = _unit_channels(np.ascontiguousarray(dense_features2, np.float32))
    in_maps = []
    mats = []
    for n in range(N_ITEMS):
        inp_full = _patches(fi[n])                       # (576, 8836)
        ref_full = _patches(fr[n])                       # (576, 8836)
        mats.append((inp_full, ref_full))
        ref_pad = np.zeros((KPAD, REF_PAD), np.float32)
        ref_pad[:K, :NPATCH] = ref_full
        ref_pad = np.ascontiguousarray(
            ref_pad.reshape(KCH, 128, REF_PAD))
        for j in range(CHUNKS_PER_ITEM):
            inp_pad = np.zeros((KPAD, CHUNK_PAD), np.float32)
            inp_pad[:K, :CHUNK] = inp_full[:, j * CHUNK:(j + 1) * CHUNK]
            inp_pad = np.ascontiguousarray(
                inp_pad.reshape(KCH, 128, CHUNK_PAD))
            in_maps.append({"inp": inp_pad, "ref": ref_pad})
    return in_maps, mats


def _combine_core(val, idx):
    # val/idx: (128, N_BLOCKS*N_STRIPS*8)
    # -> (CHUNK,) global ref argmax, (CHUNK,) top1-top2 margin
    v8 = val.reshape(128, N_BLOCKS, N_STRIPS, 8)
    v8 = v8.transpose(1, 0, 2, 3).reshape(CHUNK_PAD, N_STRIPS * 8)[:CHUNK]
    v = v8[:, ::8].reshape(CHUNK, N_STRIPS)       # per-strip top-1
    ix = idx.reshape(128, N_BLOCKS, N_STRIPS, 8)[..., 0].astype(np.int64)
    ix = ix.transpose(1, 0, 2).reshape(CHUNK_PAD, N_STRIPS)[:CHUNK]
    g = ix + (np.arange(N_STRIPS, dtype=np.int64) * STRIP)[None, :]
    sel = np.argmax(v, axis=1)            # first occurrence == earliest strip
    top2 = np.partition(v8, N_STRIPS * 8 - 2, axis=1)[:, -2:]
    margin = top2[:, 1] - top2[:, 0]
    return g[np.arange(CHUNK), sel], margin


def _flow_output(max_idx):
    # max_idx: (NPATCH,) int -> (18, H, W) float32, mirroring the reference
    mi = max_idx.reshape(HP, WP)
    fw = (mi % WP).astype(np.float32) - np.arange(WP, dtype=np.float32)[None, :]
    fh = (mi // WP).astype(np.float32) - np.arange(HP, dtype=np.float32)[:, None]
    flow = np.stack([fw, fh], axis=-1)                     # (94, 94, 2)
    flow = np.pad(flow, ((0, PS - 1), (0, PS - 1), (0, 0)))  # (96, 96, 2)
    shifted = np.stack([np.pad(flow, ((i, 0), (j, 0), (0, 0)))[:H, :W]
                        for i in range(PS) for j in range(PS)], axis=0)
    out = np.stack([shifted[..., 1], shifted[..., 0]], axis=1)  # (9, 2, H, W)
    return out.reshape(2 * PS * PS, H, W).astype(np.float32)


def kernel(dense_features1, dense_features2):
    from concourse import bass_utils

    nc = _get_nc()
    in_maps, mats = _prep_inputs(dense_features1, dense_features2)
    res = bass_utils.run_bass_kernel_spmd(
        nc, in_maps, core_ids=list(range(N_CORES)))
    out = np.empty((N_ITEMS, 2 * PS * PS, H, W), np.float32)
    for n in range(N_ITEMS):
        parts = [
            _combine_core(res.results[n * CHUNKS_PER_ITEM + j]["val"],
                          res.results[n * CHUNKS_PER_ITEM + j]["idx"])
            for j in range(CHUNKS_PER_ITEM)
        ]
        max_idx = np.concatenate([p[0] for p in parts])
        margin = np.concatenate([p[1] for p in parts])
        flagged = np.flatnonzero(margin < MARGIN_THRESH)
        if flagged.size:
            inp_full, ref_full = mats[n]
            corr = inp_full[:, flagged].T.astype(np.float64) @ \
                ref_full.astype(np.float64)
            max_idx[flagged] = np.argmax(corr, axis=1)
        out[n] = _flow_output(max_idx)
    return out


# revision 9
# speedup vs baseline: 1.1251x; 1.0213x over previous
"""CorrespondenceGeneration kernel for 8 TRN2 NeuronCores.

Reference computation (per item): unit-normalize features over channels,
build 3x3 patch matrices, corr = inp_patches^T @ ref_patches, argmax over
ref patches (first occurrence on ties), convert argmax index to flow,
9 tensor-shifts, channel reorder.

Sharding: core c -> (item = c//4, n_in chunk = c%4 of 2209 rows). Each core
computes its corr rows against ALL ref patches, streamed in 6 column strips
of 1536; per strip the DVE max/max_index ops produce (top1 val, first idx)
per row. Host combines strips (strict >, ascending strip order == exact
first-occurrence-tie argmax) and does the cheap index->flow postprocessing.

Note: the reference's per-patch-column normalization of ref divides every
column by ||col||+eps with ||col|| == 3 exactly (9 unit-norm pixels), a
global positive scale that argmax is invariant to -- so it is skipped.
"""

import sys

if "/opt/trn_rl_repo" not in sys.path:
    sys.path.insert(0, "/opt/trn_rl_repo")

import numpy as np

# ---- problem constants (hardcoded; kernel.py must be self-contained) ----
N_ITEMS = 2
C = 64
H = W = 96
PS = 3
HP = WP = H - PS + 1          # 94
NPATCH = HP * WP              # 8836
K = C * PS * PS               # 576
KPAD = 640                    # 5 x 128
KCH = 5                       # K chunks of 128
N_CORES = 8
CHUNKS_PER_ITEM = 4
CHUNK = NPATCH // CHUNKS_PER_ITEM      # 2209
CHUNK_PAD = 2304                       # 18 x 128
N_BLOCKS = CHUNK_PAD // 128            # 18
REF_PAD = 9216                         # 4 x 2048 + 1024
STRIP = 2048                           # 4 PSUM banks of 512 fp32
N_STRIPS = 5                           # strip widths: 2048 x 4, 1024 (pad)
LAST_W = REF_PAD - (N_STRIPS - 1) * STRIP      # 1024
LAST_REAL = NPATCH - (N_STRIPS - 1) * STRIP    # 644 real cols in last strip
EPS_NORMALIZE = 1e-12

# matmul input dtype: "float32" (exact, 4 cyc/row) or "float32r" (1 cyc/row)
MM_DTYPE = "float32r"
# float32r matmul error is ~2.6e-4 max (measured); rows whose top-2 corr gap
# is below this threshold get an exact fp64 rescore on the host (~700 rows).
MARGIN_THRESH = 4e-3

_COMPILED = {}


def _build_module():
    import concourse.bacc as bacc
    from concourse.tile import TileContext
    from concourse import mybir

    dt_mm = getattr(mybir.dt, MM_DTYPE)
    nc = bacc.Bacc("TRN2", target_bir_lowering=False, debug=False,
                   num_devices=N_CORES)
    inp_d = nc.dram_tensor("inp", [KCH, 128, CHUNK_PAD], dt_mm,
                           kind="ExternalInput").ap()
    ref_d = nc.dram_tensor("ref", [KCH, 128, REF_PAD], dt_mm,
                           kind="ExternalInput").ap()
    NSLOT = N_BLOCKS * N_STRIPS            # 108
    val_d = nc.dram_tensor("val", [128, NSLOT * 8], mybir.dt.float32,
                           kind="ExternalOutput").ap()
    idx_d = nc.dram_tensor("idx", [128, NSLOT * 8], mybir.dt.uint32,
                           kind="ExternalOutput").ap()

    # strip s geometry: width on PSUM/PE (zero-padded), real cols for DVE scan
    # (a <256-wide matmul falls off float32r's fast path, so PE computes the
    # zero-padded width and only the DVE scan is trimmed)
    widths = [STRIP] * (N_STRIPS - 1) + [LAST_W]
    reals = [STRIP] * (N_STRIPS - 1) + [LAST_REAL]

    with TileContext(nc) as tc:
        with tc.tile_pool(name="inp", bufs=1) as inp_pool, \
             tc.tile_pool(name="ref", bufs=2) as ref_pool, \
             tc.tile_pool(name="corr", bufs=4) as corr_pool, \
             tc.tile_pool(name="acc", bufs=1) as acc_pool, \
             tc.tile_pool(name="psum", bufs=2, space="PSUM") as psum_pool:
            # first ref strip in fine (k, 512-col) slices, before the big inp
            # transfer, so the first matmuls are gated on ~KB not MB of DMA
            ref_tiles = {}
            ref_tiles[0] = ref_pool.tile([128, KCH * STRIP], dt_mm,
                                         tag="ref", name="ref_sb0")
            for k in range(KCH):
                for j in range(STRIP // 512):
                    nc.sync.dma_start(
                        ref_tiles[0][:, k * STRIP + j * 512:
                                     k * STRIP + (j + 1) * 512],
                        ref_d[k, :, j * 512:(j + 1) * 512])
            # inp in (k, 6-block) slices: block 0 only needs the first slice
            inp_sb = inp_pool.tile([128, KCH * CHUNK_PAD], dt_mm)
            for g in range(3):
                for k in range(KCH):
                    lo, hi = g * 768, (g + 1) * 768
                    nc.sync.dma_start(
                        inp_sb[:, k * CHUNK_PAD + lo:k * CHUNK_PAD + hi],
                        inp_d[k, :, lo:hi])
            acc_val = acc_pool.tile([128, NSLOT * 8], mybir.dt.float32)
            acc_idx = acc_pool.tile([128, NSLOT * 8], mybir.dt.uint32)
            for s in range(N_STRIPS):
                w, real = widths[s], reals[s]
                if s not in ref_tiles:
                    ref_tiles[s] = ref_pool.tile([128, KCH * STRIP], dt_mm,
                                                 tag="ref", name=f"ref_sb{s}")
                    for k in range(KCH):
                        nc.sync.dma_start(
                            ref_tiles[s][:, k * STRIP:k * STRIP + w],
                            ref_d[k, :, s * STRIP:s * STRIP + w])
                ref_sb = ref_tiles[s]
                for b in range(N_BLOCKS):
                    pt = psum_pool.tile([128, STRIP], mybir.dt.float32,
                                        tag="pt", name=f"pt_{s}_{b}")
                    for k in range(KCH):
                        for j in range(w // 512):
                            nc.tensor.matmul(
                                pt[:, j * 512:(j + 1) * 512],
                                inp_sb[:, k * CHUNK_PAD + b * 128:
                                       k * CHUNK_PAD + (b + 1) * 128],
                                ref_sb[:, k * STRIP + j * 512:
                                       k * STRIP + (j + 1) * 512],
                                start=(k == 0), stop=(k == KCH - 1))
                    ct = corr_pool.tile([128, STRIP], mybir.dt.float32)
                    nc.scalar.copy(ct[:, :w], pt[:, :w])
                    slot = (s * N_BLOCKS + b) * 8
                    nc.vector.max(acc_val[:, slot:slot + 8], ct[:, :real])
                    nc.vector.max_index(acc_idx[:, slot:slot + 8],
                                        acc_val[:, slot:slot + 8],
                                        ct[:, :real])
                # stream this strip's results out as soon as they are done
                lo, hi = s * N_BLOCKS * 8, (s + 1) * N_BLOCKS * 8
                nc.sync.dma_start(val_d[:, lo:hi], acc_val[:, lo:hi])
                nc.sync.dma_start(idx_d[:, lo:hi], acc_idx[:, lo:hi])

    nc.compile()
    return nc


def _get_nc():
    if "nc" not in _COMPILED:
        _COMPILED["nc"] = _build_module()
    return _COMPILED["nc"]


def _unit_channels(f):
    # f: (N, C, H, W) float32; unit L2 norm over channels per pixel
    n = np.sqrt(np.sum(f * f, axis=1, keepdims=True, dtype=np.float32))
    return (f / np.maximum(n, EPS_NORMALIZE)).astype(np.float32)


def _patches(f):
    # f: (C, H, W) -> (K, NPATCH), row index = c*9 + dy*3 + dx
    out = np.empty((C, PS * PS, HP, WP), np.float32)
    for dy in range(PS):
        for dx in range(PS):
            out[:, dy * PS + dx] = f[:, dy:dy + HP, dx:dx + WP]
    return out.reshape(K, NPATCH)


def _prep_inputs(dense_features1, dense_features2):
    fi = _unit_channels(np.ascontiguousarray(dense_features1, np.float32))
    fr = _unit_channels(np.ascontiguousarray(dense_features2, np.float32))
    in_maps = []
    mats = []
    for n in range(N_ITEMS):
        inp_full = _patches(fi[n])                       # (576, 8836)
        ref_full = _patches(fr[n])                       # (576, 8836)
        mats.append((inp_full, ref_full))
        ref_pad = np.zeros((KPAD, REF_PAD), np.float32)
        ref_pad[:K, :NPATCH] = ref_full
        ref_pad = np.ascontiguousarray(
            ref_pad.reshape(KCH, 128, REF_PAD))
        for j in range(CHUNKS_PER_ITEM):
            inp_pad = np.zeros((KPAD, CHUNK_PAD), np.float32)
            inp_pad[:K, :CHUNK] = inp_full[:, j * CHUNK:(j + 1) * CHUNK]
            inp_pad = np.ascontiguousarray(
                inp_pad.reshape(KCH, 128, CHUNK_PAD))
            in_maps.append({"inp": inp_pad, "ref": ref_pad})
    return in_maps, mats


def _combine_core(val, idx):
    # val/idx: (128, N_STRIPS*N_BLOCKS*8), slot = (s*N_BLOCKS + b)*8
    # -> (CHUNK,) global ref argmax, (CHUNK,) top1-top2 margin
    v8 = val.reshape(128, N_STRIPS, N_BLOCKS, 8)
    v8 = v8.transpose(2, 0, 1, 3).reshape(CHUNK_PAD, N_STRIPS * 8)[:CHUNK]
    v = v8[:, 0::8]                               # per-strip top-1
    ix = idx.reshape(128, N_STRIPS, N_BLOCKS, 8)[..., 0].astype(np.int64)
    ix = ix.transpose(2, 0, 1).reshape(CHUNK_PAD, N_STRIPS)[:CHUNK]
    g = ix + (np.arange(N_STRIPS, dtype=np.int64) * STRIP)[None, :]
    sel = np.argmax(v, axis=1)            # first occurrence == earliest strip
    top2 = np.partition(v8, N_STRIPS * 8 - 2, axis=1)[:, -2:]
    margin = top2[:, 1] - top2[:, 0]
    return g[np.arange(CHUNK), sel], margin


def _flow_output(max_idx):
    # max_idx: (NPATCH,) int -> (18, H, W) float32, mirroring the reference
    mi = max_idx.reshape(HP, WP)
    fw = (mi % WP).astype(np.float32) - np.arange(WP, dtype=np.float32)[None, :]
    fh = (mi // WP).astype(np.float32) - np.arange(HP, dtype=np.float32)[:, None]
    flow = np.stack([fw, fh], axis=-1)                     # (94, 94, 2)
    flow = np.pad(flow, ((0, PS - 1), (0, PS - 1), (0, 0)))  # (96, 96, 2)
    shifted = np.stack([np.pad(flow, ((i, 0), (j, 0), (0, 0)))[:H, :W]
                        for i in range(PS) for j in range(PS)], axis=0)
    out = np.stack([shifted[..., 1], shifted[..., 0]], axis=1)  # (9, 2, H, W)
    return out.reshape(2 * PS * PS, H, W).astype(np.float32)


def kernel(dense_features1, dense_features2):
    from concourse import bass_utils

    nc = _get_nc()
    in_maps, mats = _prep_inputs(dense_features1, dense_features2)
    res = bass_utils.run_bass_kernel_spmd(
        nc, in_maps, core_ids=list(range(N_CORES)))
    out = np.empty((N_ITEMS, 2 * PS * PS, H, W), np.float32)
    for n in range(N_ITEMS):
        parts = [
            _combine_core(res.results[n * CHUNKS_PER_ITEM + j]["val"],
                          res.results[n * CHUNKS_PER_ITEM + j]["idx"])
            for j in range(CHUNKS_PER_ITEM)
        ]
        max_idx = np.concatenate([p[0] for p in parts])
        margin = np.concatenate([p[1] for p in parts])
        flagged = np.flatnonzero(margin < MARGIN_THRESH)
        if flagged.size:
            inp_full, ref_full = mats[n]
            corr = inp_full[:, flagged].T.astype(np.float64) @ \
                ref_full.astype(np.float64)
            max_idx[flagged] = np.argmax(corr, axis=1)
        out[n] = _flow_output(max_idx)
    return out
